# revision 43
# baseline (speedup 1.0000x reference)
"""Trainium2 Bass kernel for nn_Concept_model_171798691895.

Model: 8-way categorical embedding -> 2-layer LSTM(H=8) over T=64 ->
tiny linear heads -> per-example scalar.  B=16384 sharded data-parallel
over 8 NeuronCores (2048 examples/core).

Device layout (per core):
  batch 2048 = 4 chunks x 512 (chunk c = examples [512c, 512c+512))
  matmul M-packing: out partition m = gt*32 + c*8 + f   (gate-type major)
      gt in {0:i, 1:f, 2:o, 3:g}; per-chunk feature f in [0,8)
  gates PSUM [128, 512] per layer; ONE sigmoid ACT covers all 128 rows:
      i/f/o rows natural (sigma); g rows pre-doubled so row value is
      sg = sigma(2g) = (tanh(g)+1)/2.
  half-state convention C2 = c/2:
      P2 = (sg - 0.5) * si          (= si*tanh(g)/2, one STT)
      Q2 = sf * C2                  (TT bf16 2x)
      C2' = P2 + Q2                 (TT bf16 2x)
      TC = tanh(2*C2')              (ACT, scale=2)
      H  = so * TC                  (TT bf16 2x; natural h)
  Recurrent ring R_s [97, NB]: [OUT(s-1) 0:32 | const 32 | zero 33:64 |
  H0(s) 64:96 | zero 96].  Layer-1 gates = ONE K=97 matmul (zero lhsT
  rows are free: matmul cost is N-bound).  Layer-0 = K=65 emb matmul +
  K=32 h matmul (PSUM accumulate).  Head (step s) = K=33 matmul on
  R_{s+1}[0:33] -> psum [36, NB] (theta 0:4, h 32:36, both UNweighted by
  w_t); PW = theta*h stored to PWH ring; every 8 steps DMA-rearranged to
  Rt [65, 2048] (partition = t); final pred = wts^T @ Rt in one matmul
  with b3 via const row 64.
"""

import os
import sys
import numpy as np

for _p in ("/opt/trn_rl_repo", os.path.expanduser("~/.axon_site/_ro/trn_rl_repo")):
    if os.path.isdir(_p) and _p not in sys.path:
        sys.path.insert(0, _p)

B, T, H = 16384, 64, 8
VOCABS = [2, 2, 21, 22, 5, 2, 22, 24]
EDIMS = [1, 1, 3, 3, 1, 1, 3, 3]
NCORE = 8
BC = B // NCORE          # 2048 per core
NCH = 4                  # chunks per core
NB = BC // NCH           # 512 batch per chunk (matmul N)
ED = 16                  # total embedding dim
KE = NCH * ED + 1        # 65: emb K rows + shared const row

LAST_EXEC_NS = None
_CACHE = {}

# torch gate order in weight rows: i(0:8) f(8:16) g(16:24) o(24:32)
# our gt order: 0:i 1:f 2:o 3:g
_WROW = {0: 0, 1: 8, 2: 24, 3: 16}


def _pack_weights(inp):
    f32 = np.float32
    W_ih0 = np.asarray(inp["W_ih0"], f32); W_hh0 = np.asarray(inp["W_hh0"], f32)
    b0 = np.asarray(inp["b_ih0"], f32) + np.asarray(inp["b_hh0"], f32)
    W_ih1 = np.asarray(inp["W_ih1"], f32); W_hh1 = np.asarray(inp["W_hh1"], f32)
    b1 = np.asarray(inp["b_ih1"], f32) + np.asarray(inp["b_hh1"], f32)

    def gsc(gt):
        # g rows doubled so sigma sees 2g
        return 2.0 if gt == 3 else 1.0

    # lt0e [KE, 128], lt0h [32, 128], lt1 [97, 128]
    lt0e = np.zeros((KE, 128), f32)
    lt0h = np.zeros((32, 128), f32)
    lt1 = np.zeros((97, 128), f32)
    for gt in range(4):
        s = gsc(gt)
        for c in range(NCH):
            for f in range(H):
                m = gt * 32 + c * 8 + f
                wr = _WROW[gt] + f
                lt0e[c * ED:(c + 1) * ED, m] = W_ih0[wr, :] * s
                lt0e[KE - 1, m] = b0[wr] * s
                lt0h[c * 8:(c + 1) * 8, m] = W_hh0[wr, :] * s
                # R layout: OUT(s-1) at 0:32, const at 32, H0(s) at 64:96
                lt1[c * 8:(c + 1) * 8, m] = W_hh1[wr, :] * s
                lt1[32, m] = b1[wr] * s
                lt1[64 + c * 8:64 + (c + 1) * 8, m] = W_ih1[wr, :] * s

    # head: theta = v6.out + s6 ; h = v1.out + s1 (NO w_t folding)
    fc6_w = np.asarray(inp["fc6_w"], f32); fc6_b = np.asarray(inp["fc6_b"], f32)
    fc7_w = np.asarray(inp["fc7_w"], f32); fc7_b = np.asarray(inp["fc7_b"], f32)
    fc1_w = np.asarray(inp["fc1_w"], f32); fc1_b = np.asarray(inp["fc1_b"], f32)
    fc2_w = np.asarray(inp["fc2_w"], f32); fc2_b = np.asarray(inp["fc2_b"], f32)
    v6 = (fc7_w @ fc6_w)[0]; s6 = float(fc6_b @ fc7_w[0]) + float(fc7_b[0])
    v1 = (fc2_w @ fc1_w)[0]; s1 = float(fc1_b @ fc2_w[0]) + float(fc2_b[0])
    # lth [33, 36]: rhs rows = [OUT | const]; theta cols 0:4, h cols 32:36
    lth = np.zeros((33, 36), f32)
    for c in range(NCH):
        lth[c * 8:(c + 1) * 8, c] = v6
        lth[32, c] = s6
        lth[c * 8:(c + 1) * 8, 32 + c] = v1
        lth[32, 32 + c] = s1
    # tail weights [65, 1]: w_t rows + b3 on const row 64
    wts = np.asarray(inp["fc3_w"], f32)[0]      # [T]
    b3 = float(np.asarray(inp["fc3_b"], f32)[0])
    wt = np.zeros((65, 1), f32)
    wt[:T, 0] = wts
    wt[64, 0] = b3
    import ml_dtypes
    bf16 = ml_dtypes.bfloat16
    return (lt0e.astype(bf16), lt0h.astype(bf16), lt1.astype(bf16),
            lth.astype(bf16), wt.astype(bf16))


def _build_ebr(inp, core):
    """Host-side embedding lookup -> per-core rhs region [KE, T*NB] bf16."""
    import ml_dtypes
    x = np.asarray(inp["x"])[core * BC:(core + 1) * BC]          # [BC, T, 8]
    cols = []
    for i in range(8):
        tab = np.asarray(inp["e" + str(i + 1)], np.float32)      # [V_i, d_i]
        cols.append(tab[x[:, :, i]])                             # [BC, T, d_i]
    emb = np.concatenate(cols, axis=2)                           # [BC, T, 16]
    emb = emb.reshape(NCH, NB, T, ED).transpose(0, 3, 2, 1)      # [c, j, t, b]
    ebr = np.ones((KE, T, NB), np.float32)
    ebr[:NCH * ED] = emb.reshape(NCH * ED, T, NB)
    return ebr.reshape(KE, T * NB).astype(ml_dtypes.bfloat16)


def _build_nc():
    import concourse.bass as bass
    import concourse.tile as tile
    from concourse import bacc, mybir

    AF = mybir.ActivationFunctionType
    OP = mybir.AluOpType
    F32 = mybir.dt.float32
    BF16 = mybir.dt.bfloat16

    nc = bacc.Bacc("TRN2", target_bir_lowering=False, debug=False,
                   num_devices=NCORE)
    ebr_ext = nc.dram_tensor("ebr", [KE, T * NB], BF16, kind="ExternalInput")
    lt0e_ext = nc.dram_tensor("lt0e", [KE, 128], BF16, kind="ExternalInput")
    lt0h_ext = nc.dram_tensor("lt0h", [32, 128], BF16, kind="ExternalInput")
    lt1_ext = nc.dram_tensor("lt1", [97, 128], BF16, kind="ExternalInput")
    lth_ext = nc.dram_tensor("lth", [33, 36], BF16, kind="ExternalInput")
    wt_ext = nc.dram_tensor("wt", [65, 1], BF16, kind="ExternalInput")
    pred_ext = nc.dram_tensor("pred", [1, BC], F32, kind="ExternalOutput")

    with tile.TileContext(nc) as tc:
        with (
            tc.tile_pool(name="persist", bufs=1) as pp,
            tc.tile_pool(name="sig", bufs=3) as sp,
            tc.tile_pool(name="work", bufs=3) as wp,
            tc.tile_pool(name="psum", bufs=3, space="PSUM") as psp,
            tc.tile_pool(name="psumh", bufs=2, space="PSUM") as psph,
        ):
            # ---- persistent tiles ----
            ebr = pp.tile([KE, T * NB], BF16)
            w0e = pp.tile([KE, 128], BF16)
            w0h = pp.tile([96, 128], BF16)   # live rows 64:96
            w1 = pp.tile([97, 128], BF16)
            whd = pp.tile([33, 36], BF16)
            wt = pp.tile([65, 1], BF16)
            CC0 = pp.tile([64, NB], BF16)    # C2 state rows 32:64
            CC1 = pp.tile([64, NB], BF16)
            R = [pp.tile([97, NB], BF16, name=f"R{i}") for i in range(3)]
            PWH = pp.tile([4, 8 * NB], BF16)
            Rt = pp.tile([65, BC], BF16)
            predS = pp.tile([1, BC], F32)

            # Startup-ordered init: the first loop matmuls need only
            # ebr slice 0 + L0 weights, so trigger those DMAs first (each
            # gpsimd dma trigger costs ~650ns of queue time); memsets and
            # late-tick weights follow.
            # graded slice widths: tiny first slices so tick 0 can
            # start as soon as ~133KB lands; wide slices later.
            cuts = [0, 2, 4, 8, 16, 24, 32, 40, 48, 56, 64]

            def ebr_slice(q):
                a, b = cuts[q] * NB, cuts[q + 1] * NB
                nc.gpsimd.dma_start(ebr[:, a:b], ebr_ext.ap()[:, a:b])

            nc.gpsimd.dma_start(w0e[:], lt0e_ext.ap())
            ebr_slice(0)
            nc.gpsimd.dma_start(w0h[64:96, :], lt0h_ext.ap())
            ebr_slice(1)
            nc.gpsimd.dma_start(w1[:], lt1_ext.ap())
            nc.gpsimd.memset(CC0[32:64, :], 0.0)
            nc.gpsimd.memset(CC1[32:64, :], 0.0)
            ebr_slice(2)
            nc.gpsimd.memset(R[0][:], 0.0)
            nc.gpsimd.memset(R[0][32:33, :], 1.0)
            nc.gpsimd.dma_start(whd[:], lth_ext.ap())
            for i in range(1, 3):
                nc.gpsimd.memset(R[i][:], 0.0)
                nc.gpsimd.memset(R[i][32:33, :], 1.0)
            for q in range(3, 10):
                ebr_slice(q)
            nc.gpsimd.dma_start(wt[:], wt_ext.ap())
            nc.gpsimd.memset(Rt[64:65, :], 1.0)

            def cell_pre(ps, CC, Sg):
                """sigma-form cell math up to the C-state update.
                ps: gates PSUM [128, NB] (i 0:32, f 32:64, o 64:96,
                2g 96:128).  CC: C2 state tile (rows 32:64).
                Walrus rule: two SBUF tensor operands of one DVE op must
                share a start partition — realign sg via a single-operand
                tensor_scalar (4x mode) so every TT pair is same-base."""
                S = sp.tile([128, NB], BF16, tag=Sg + "S")
                nc.scalar.activation(S[:], ps[:], AF.Sigmoid)
                G = wp.tile([32, NB], BF16, tag=Sg + "G")
                nc.vector.tensor_scalar_add(G[:], S[96:128, :], -0.5)
                TP = wp.tile([32, NB], BF16, tag=Sg + "P")
                nc.vector.tensor_tensor(
                    out=TP[:], in0=G[:], in1=S[0:32, :], op=OP.mult)
                TQ = wp.tile([32, NB], BF16, tag=Sg + "Q")
                nc.vector.tensor_tensor(
                    out=TQ[:], in0=S[32:64, :], in1=CC[32:64, :],
                    op=OP.mult)
                nc.vector.tensor_tensor(
                    out=CC[32:64, :], in0=TP[:], in1=TQ[:], op=OP.add)
                return S

            def cell_tanh(CC, Sg):
                TC = wp.tile([96, NB], BF16, tag=Sg + "C")
                nc.scalar.activation(TC[64:96, :], CC[32:64, :],
                                     AF.Tanh, scale=2.0)
                return TC

            # Emission order per tick is the scheduler priority.  Keep the
            # scalar queue [s1, s0, COPY, A3_1, A3_0] and the DVE queue
            # [L1-pre, L0-pre, V4_1, Vh, V4_0] so the critical C-state
            # cycles close without cross-chain queue races (the V4 OUT/H0
            # writes and the head product fill the A3 latencies).
            for t in range(T + 3):
                s = t - 2       # layer-1 step
                hs = t - 3      # head step
                do1 = 0 <= s < T
                do0 = t < T
                doh = 0 <= hs < T
                S1 = S0 = psh = HC = None
                # ---- layer 1, step s: gates + pre-tanh ----
                if do1:
                    Rs = R[s % 3]
                    ps1 = psp.tile([128, NB], F32, tag="ps1")
                    nc.tensor.matmul(ps1[:], w1[:], Rs[0:97, :],
                                     start=True, stop=True)
                    S1 = cell_pre(ps1, CC1, "a")
                # ---- layer 0, step t: gates + pre-tanh ----
                if do0:
                    sl = slice(t * NB, (t + 1) * NB)
                    ps0 = psp.tile([128, NB], F32, tag="ps0")
                    nc.tensor.matmul(ps0[:], w0e[:], ebr[:, sl],
                                     start=True, stop=(t == 0))
                    if t > 0:
                        nc.tensor.matmul(ps0[:], w0h[64:96, :],
                                         R[(t - 1) % 3][64:96, :],
                                         start=False, stop=True)
                    S0 = cell_pre(ps0, CC0, "b")
                # ---- head matmul + h-copy (fills scalar gap) ----
                if doh:
                    Rh = R[(hs + 1) % 3]
                    psh = psph.tile([36, NB], F32, tag="psh")
                    nc.tensor.matmul(psh[:], whd[:], Rh[0:33, :],
                                     start=True, stop=True)
                    HC = wp.tile([4, NB], F32, tag="HC")
                    nc.scalar.activation(HC[:], psh[32:36, :], AF.Copy)
                # ---- tanh + H writes, layer 1 first ----
                if do1:
                    TC1 = cell_tanh(CC1, "a")
                    nc.vector.tensor_tensor(
                        out=R[(s + 1) % 3][0:32, :], in0=S1[64:96, :],
                        in1=TC1[64:96, :], op=OP.mult)
                if do0:
                    TC0 = cell_tanh(CC0, "b")
                if doh:   # head product sits between V4_1 and V4_0
                    slot = hs % 8
                    nc.vector.tensor_tensor(
                        out=PWH[:, slot * NB:(slot + 1) * NB],
                        in0=psh[0:4, :], in1=HC[:], op=OP.mult)
                if do0:   # H0(t) -> R_t[64:96]
                    nc.vector.tensor_tensor(
                        out=R[t % 3][64:96, :], in0=S0[64:96, :],
                        in1=TC0[64:96, :], op=OP.mult)
                if doh and hs % 8 == 7:
                    t0 = hs - 7
                    for c in range(NCH):
                        nc.gpsimd.dma_start(
                            Rt[t0:t0 + 8, c * NB:(c + 1) * NB],
                            PWH[c:c + 1, :])

            # ---------------- tail reduce over t ----------------
            for q in range(NCH):
                pt = psph.tile([1, NB], F32, tag="psh")
                nc.tensor.matmul(pt[:], wt[0:65, :],
                                 Rt[0:65, q * NB:(q + 1) * NB],
                                 start=True, stop=True)
                nc.scalar.activation(predS[:, q * NB:(q + 1) * NB], pt[:],
                                     AF.Copy)
            nc.gpsimd.dma_start(pred_ext.ap(), predS[:])

    nc.compile()
    return nc


def kernel(**inputs):
    global LAST_EXEC_NS
    from concourse.bass_utils import run_bass_kernel_spmd

    wpack = _pack_weights(inputs)
    key = "nc"
    if key not in _CACHE:
        _CACHE[key] = _build_nc()
    nc = _CACHE[key]

    lt0e, lt0h, lt1, lth, wt = wpack
    in_maps = []
    for core in range(NCORE):
        in_maps.append({
            "ebr": np.ascontiguousarray(_build_ebr(inputs, core)),
            "lt0e": lt0e, "lt0h": lt0h, "lt1": lt1, "lth": lth, "wt": wt,
        })
    trace = bool(int(os.environ.get("BASS_KERNEL_TRACE", "0")))
    if trace:
        try:
            import tracehook
            tracehook.install()
        except Exception:
            pass
    res = run_bass_kernel_spmd(nc, in_maps, core_ids=list(range(NCORE)),
                               trace=trace)
    LAST_EXEC_NS = res.exec_time_ns
    out = np.empty((B, 1), np.float32)
    for core in range(NCORE):
        out[core * BC:(core + 1) * BC, 0] = res.results[core]["pred"][0]
    return out
